# revision 7
# baseline (speedup 1.0000x reference)
"""COMA loss kernel for Trainium2 (8 NeuronCores, data-parallel over batch).

v2: fp16 streaming path. Host marshals logit/q/target_q per core into
[BA, T*N] fp16 (contiguous per-partition rows -> efficient DMA + fp16
2x DVE modes). Per (t,b,a) sums over N are computed as halving add
trees on DVE (2x fp16 TT) instead of 1x-only TENSOR_REDUCE; the onehot
build and two of the six reductions run on GpSimd, which never
contends with DVE for SBUF ports on TT/reduce ops. Scalar engine does
exp. Stage-2 per-(t,b,a) math and the lambda-return scan stay fp32.

Per-core partials [128,3] are summed on host (the "all-reduce").
"""

import sys

for _p in ("/opt/trn_rl_repo",):
    if _p not in sys.path:
        sys.path.insert(0, _p)

import numpy as np

import concourse.bass as bass
import concourse.bacc as bacc
import concourse.mybir as mybir
from concourse.bass_utils import run_bass_kernel_spmd
from concourse.tile import TileContext

T, B, A, N = 256, 128, 8, 64
M = 8                 # cores
BL = B // M           # local batch
BA = BL * A           # 128 rows -> partition dim
TC = 64               # t-chunk size
NCHUNK = T // TC
GAMMA, LAMBDA = 0.99, 0.95

F32 = mybir.dt.float32
F16 = mybir.dt.float16


def _tree_reduce(nc, pr, out_sl, tc):
    """Sum pr[BA, tc, N] over N into out_sl[BA, tc] (f32) via halving adds.

    Levels at fp16 2x; the last (fp16 in -> f32 out) is 1x but tiny.
    """
    w = N
    while w > 2:
        h = w // 2
        nc.vector.tensor_add(pr[:, :, 0:h], pr[:, :, 0:h], pr[:, :, h:w])
        w = h
    nc.vector.tensor_tensor(
        out=out_sl, in0=pr[:, :, 0], in1=pr[:, :, 1], op=mybir.AluOpType.add
    )


def build_program() -> bass.Bass:
    nc = bacc.Bacc("TRN2", target_bir_lowering=False, debug=False)

    logit = nc.dram_tensor("logit", [BA, T * N], F16, kind="ExternalInput")
    qv = nc.dram_tensor("qv", [BA, T * N], F16, kind="ExternalInput")
    tqv = nc.dram_tensor("tqv", [BA, T * N], F16, kind="ExternalInput")
    act = nc.dram_tensor("act", [BA, T], F16, kind="ExternalInput")
    wgt = nc.dram_tensor("wgt", [BA, T], F32, kind="ExternalInput")
    rwd = nc.dram_tensor("rwd", [BA, T], F32, kind="ExternalInput")
    out = nc.dram_tensor("out", [BA, 3], F32, kind="ExternalOutput")

    OP = mybir.AluOpType

    with TileContext(nc) as tc:
        with (
            tc.tile_pool(name="inp", bufs=2) as inp,
            tc.tile_pool(name="scr", bufs=2) as scr,
            tc.tile_pool(name="per", bufs=1) as per,
        ):
            # constants / small inputs
            iota_i = per.tile([BA, N], mybir.dt.int32)
            nc.gpsimd.iota(iota_i[:], pattern=[[1, N]], base=0, channel_multiplier=0)
            iota_f = per.tile([BA, N], F16)
            nc.vector.tensor_copy(iota_f[:], iota_i[:])

            act_t = per.tile([BA, T], F16)
            nc.sync.dma_start(out=act_t[:], in_=act[:])
            w_t = per.tile([BA, T], F32)
            nc.sync.dma_start(out=w_t[:], in_=wgt[:])
            r_t = per.tile([BA, T], F32)
            nc.sync.dma_start(out=r_t[:], in_=rwd[:])

            # per-(t,ba) sums, [128, T] f32
            sum_e = per.tile([BA, T], F32)
            dot_eq = per.tile([BA, T], F32)
            dot_el = per.tile([BA, T], F32)
            q_tk = per.tile([BA, T], F32)
            tq_tk = per.tile([BA, T], F32)
            l_tk = per.tile([BA, T], F32)

            # ---- stage 1: streamed over t-chunks -------------------------
            for c in range(NCHUNK):
                t0 = c * TC
                sl = slice(t0, t0 + TC)
                fsl = slice(t0 * N, (t0 + TC) * N)

                lg = inp.tile([BA, TC, N], F16, tag="lg")
                qt = inp.tile([BA, TC, N], F16, tag="qt")
                tq = inp.tile([BA, TC, N], F16, tag="tq")
                nc.sync.dma_start(out=lg[:], in_=logit[:, fsl])
                nc.sync.dma_start(out=qt[:], in_=qv[:, fsl])
                nc.sync.dma_start(out=tq[:], in_=tqv[:, fsl])

                # e = exp(logit): |logit| <= ~6, exp <= ~400 fits fp16
                e = scr.tile([BA, TC, N], F16, tag="e")
                nc.scalar.activation(
                    out=e[:], in_=lg[:], func=mybir.ActivationFunctionType.Exp
                )

                # onehot (1x on DVE: the act operand broadcasts along N)
                oh = scr.tile([BA, TC, N], F16, tag="oh")
                nc.vector.tensor_tensor(
                    out=oh[:],
                    in0=iota_f[:].unsqueeze(1).to_broadcast([BA, TC, N]),
                    in1=act_t[:, sl].unsqueeze(2).to_broadcast([BA, TC, N]),
                    op=OP.is_equal,
                )

                # Products that read e come first; then e is tree-reduced
                # in place for sum_e. The gtq product runs on GpSimd
                # (frees one DVE mul; TT ops never contend with DVE for
                # SBUF ports).
                pel = scr.tile([BA, TC, N], F16, tag="pel")
                nc.vector.tensor_mul(pel[:], e[:], lg[:])
                _tree_reduce(nc, pel, dot_el[:, sl], TC)

                peq = scr.tile([BA, TC, N], F16, tag="peq")
                nc.vector.tensor_mul(peq[:], e[:], qt[:])
                _tree_reduce(nc, peq, dot_eq[:, sl], TC)

                _tree_reduce(nc, e, sum_e[:, sl], TC)

                gq = scr.tile([BA, TC, N], F16, tag="gq")
                nc.vector.tensor_mul(gq[:], oh[:], qt[:])
                _tree_reduce(nc, gq, q_tk[:, sl], TC)

                gtq = scr.tile([BA, TC, N], F16, tag="gtq")
                nc.vector.tensor_mul(gtq[:], oh[:], tq[:])
                _tree_reduce(nc, gtq, tq_tk[:, sl], TC)

                glg = scr.tile([BA, TC, N], F16, tag="glg")
                nc.vector.tensor_mul(glg[:], oh[:], lg[:])
                _tree_reduce(nc, glg, l_tk[:, sl], TC)

            # ---- stage 2: per-(t,ba) scalar math on [128, T] f32 ---------
            z = per.tile([BA, T], F32)  # logsumexp
            nc.scalar.activation(
                out=z[:], in_=sum_e[:], func=mybir.ActivationFunctionType.Ln
            )
            rs = per.tile([BA, T], F32)  # 1/sum_e
            nc.vector.reciprocal(rs[:], sum_e[:])

            logp = per.tile([BA, T], F32)
            nc.vector.tensor_tensor(out=logp[:], in0=l_tk[:], in1=z[:], op=OP.subtract)
            bl = per.tile([BA, T], F32)  # baseline = dot_eq / sum_e
            nc.vector.tensor_mul(bl[:], dot_eq[:], rs[:])
            adv = per.tile([BA, T], F32)
            nc.vector.tensor_tensor(out=adv[:], in0=q_tk[:], in1=bl[:], op=OP.subtract)
            ent = per.tile([BA, T], F32)  # entropy = z - dot_el / sum_e
            nc.vector.tensor_mul(ent[:], dot_el[:], rs[:])
            nc.vector.tensor_tensor(out=ent[:], in0=z[:], in1=ent[:], op=OP.subtract)

            pol = per.tile([BA, T], F32)  # logp * adv * w
            nc.vector.tensor_mul(pol[:], logp[:], adv[:])
            nc.vector.tensor_mul(pol[:], pol[:], w_t[:])
            entw = per.tile([BA, T], F32)
            nc.vector.tensor_mul(entw[:], ent[:], w_t[:])

            # lambda returns: ret[t] = d[t] + g*l*ret[t+1] scanned in
            # reverse time; d[t] = r[t] + g*(1-l)*tq_taken[t+1].
            d = per.tile([BA, T - 1], F32)
            nc.vector.tensor_scalar_mul(d[:], tq_tk[:, 1:T], GAMMA * (1.0 - LAMBDA))
            nc.vector.tensor_add(d[:], d[:], r_t[:, 0 : T - 1])
            gl = per.tile([BA, 1], F32)
            nc.vector.memset(gl[:], GAMMA * LAMBDA)
            ret = per.tile([BA, T - 1], F32)
            nc.vector.tensor_tensor_scan(
                out=ret[:, ::-1],
                data0=gl[:].to_broadcast([BA, T - 1]),
                data1=d[:, ::-1],
                initial=tq_tk[:, T - 1 : T],
                op0=OP.mult,
                op1=OP.add,
            )

            qd = per.tile([BA, T - 1], F32)
            nc.vector.tensor_tensor(
                out=qd[:], in0=ret[:], in1=q_tk[:, 0 : T - 1], op=OP.subtract
            )
            nc.vector.tensor_mul(qd[:], qd[:], qd[:])
            nc.vector.tensor_mul(qd[:], qd[:], w_t[:, 0 : T - 1])

            partials = per.tile([BA, 3], F32)
            AX = mybir.AxisListType.X
            nc.vector.reduce_sum(out=partials[:, 0:1], in_=pol[:], axis=AX)
            nc.vector.reduce_sum(out=partials[:, 1:2], in_=qd[:], axis=AX)
            nc.vector.reduce_sum(out=partials[:, 2:3], in_=entw[:], axis=AX)
            nc.sync.dma_start(out=out[:], in_=partials[:])

    return nc


def make_in_maps(logit, action, q_value, target_q_value, reward, weight):
    """Shard + marshal full inputs into per-core input dicts."""
    logit = np.asarray(logit, np.float32)
    q_value = np.asarray(q_value, np.float32)
    target_q_value = np.asarray(target_q_value, np.float32)
    action = np.asarray(action)
    reward = np.asarray(reward, np.float32)
    weight = np.asarray(weight, np.float32)

    in_maps = []
    for r in range(M):
        bs, be = r * BL, (r + 1) * BL
        # [T, BL, A, N] -> [BA, T, N] fp16 contiguous
        def big(x):
            return np.ascontiguousarray(
                x[:, bs:be].reshape(T, BA, N).transpose(1, 0, 2)
            ).reshape(BA, T * N).astype(np.float16)

        in_maps.append(
            {
                "logit": big(logit),
                "qv": big(q_value),
                "tqv": big(target_q_value),
                "act": np.ascontiguousarray(
                    action[:, bs:be].reshape(T, BA).T
                ).astype(np.float16),
                "wgt": np.ascontiguousarray(weight[:, bs:be].reshape(T, BA).T),
                "rwd": np.ascontiguousarray(
                    np.repeat(reward[:, bs:be], A, axis=1).T
                ),
            }
        )
    return in_maps


def combine_partials(partials_per_core):
    """[M][128,3] partial sums -> the three scalar losses."""
    s = np.stack(partials_per_core).astype(np.float64).sum(axis=(0, 1))
    policy_loss = np.float32(-s[0] / (T * B * A))
    q_value_loss = np.float32(s[1] / ((T - 1) * B * A))
    entropy_loss = np.float32(s[2] / (T * B * A))
    return policy_loss, q_value_loss, entropy_loss


_program_cache = {}


def _get_program() -> bass.Bass:
    if "nc" not in _program_cache:
        nc = build_program()
        nc.finalize()
        _program_cache["nc"] = nc
    return _program_cache["nc"]


def kernel(logit, action, q_value, target_q_value, reward, weight):
    nc = _get_program()
    in_maps = make_in_maps(logit, action, q_value, target_q_value, reward, weight)
    res = run_bass_kernel_spmd(nc, in_maps, list(range(M))).results
    return combine_partials([np.asarray(res[i]["out"]) for i in range(M)])


# revision 9
# speedup vs baseline: 1.1680x; 1.1680x over previous
"""COMA loss kernel for Trainium2 — v4: N-on-partition + j-major free dim.

Layout per core (B sharded 8 ways, BL=16, BA=BL*A=128 rows):
  ba = 64*h + j  (h in {0,1}, j in [0,64))
  SBUF partition p = 64*h + n   (n = action index, N=64)
  free index     f = j*T + t    (F = 64*T = 16384), j-major!

All six per-(ba,t) sums over n are PE ones-matmuls (accumulating six
[128,12] one-column stationaries into one [12,512] PSUM tile). Because
f is j-major, the [12, F] sum rows convert to the stage-2 layout
s2d[j, 12, T] with a single strided SBUF->SBUF DMA per chunk (512B
contiguous segments) — no PE transposes at all. Stage-1 streams over
j-blocks of 8 (chunks are f-contiguous). The onehot is a 4x-mode
tensor_scalar is_equal against a per-partition iota. DVE carries only
the five fp16 products plus a compact merged-h stage 2.
"""

import sys

for _p in ("/opt/trn_rl_repo",):
    if _p not in sys.path:
        sys.path.insert(0, _p)

import numpy as np

import concourse.bass as bass
import concourse.bacc as bacc
import concourse.mybir as mybir
from concourse.bass_utils import run_bass_kernel_spmd
from concourse.tile import TileContext

T, B, A, N = 256, 128, 8, 64
M = 8                 # cores
BL = B // M
BA = BL * A           # 128
H, J = 2, 64          # ba = 64h + j
F = J * T             # 16384: f = j*T + t
JCH = 8               # j per chunk
NCH = J // JCH        # 8 chunks
FCH = JCH * T         # 2048
SUB = 512             # matmul f-subchunk (one PSUM bank)
NSUB = FCH // SUB     # 4
GAMMA, LAMBDA = 0.99, 0.95

F32 = mybir.dt.float32
F16 = mybir.dt.float16

# reduction-row order within [12, f]: row = 2*rho + h
R_SUME, R_DOTEQ, R_DOTEL, R_QTK, R_TQTK, R_LTK = range(6)


def build_program() -> bass.Bass:
    nc = bacc.Bacc("TRN2", target_bir_lowering=False, debug=False)

    lg_d = nc.dram_tensor("logit", [BA, F], F16, kind="ExternalInput")
    qv_d = nc.dram_tensor("qv", [BA, F], F16, kind="ExternalInput")
    tqv_d = nc.dram_tensor("tqv", [BA, F], F16, kind="ExternalInput")
    actr_d = nc.dram_tensor("actr", [BA, F], F16, kind="ExternalInput")
    iota_d = nc.dram_tensor("iotac", [BA, 1], F32, kind="ExternalInput")
    wred_d = nc.dram_tensor("wred", [BA, 6 * 12], F16, kind="ExternalInput")
    wgt_d = nc.dram_tensor("wgt", [J, H * T], F32, kind="ExternalInput")
    rwd_d = nc.dram_tensor("rwd", [J, H * T], F32, kind="ExternalInput")
    out_d = nc.dram_tensor("out", [J, H, 3], F32, kind="ExternalOutput")

    OP = mybir.AluOpType
    AX = mybir.AxisListType.X

    with TileContext(nc) as tc:
        with (
            tc.tile_pool(name="inp", bufs=2) as inp,
            tc.tile_pool(name="scr", bufs=2) as scr,
            tc.tile_pool(name="sums", bufs=2) as sums,
            tc.tile_pool(name="per", bufs=1) as per,
            tc.tile_pool(name="ps_red", bufs=3, space=bass.MemorySpace.PSUM) as ps_red,
            tc.tile_pool(name="drb", bufs=2, space="DRAM") as drb,
        ):
            # ---- constants / small inputs ---------------------------------
            iota_c = per.tile([BA, 1], F32)
            nc.sync.dma_start(out=iota_c[:], in_=iota_d[:])
            wred = per.tile([BA, 6, 12], F16)
            nc.sync.dma_start(out=wred[:], in_=wred_d[:])
            w_t = per.tile([J, H, T], F32)
            nc.sync.dma_start(out=w_t[:], in_=wgt_d[:])
            r_t = per.tile([J, H, T], F32)
            nc.sync.dma_start(out=r_t[:], in_=rwd_d[:])
            act_rep = per.tile([BA, F], F16)
            nc.sync.dma_start(out=act_rep[:], in_=actr_d[:])

            # s2d[j, r, t]: per-(ba,t) sums in stage-2 layout, r = 2*rho+h
            s2d = per.tile([J, 12, T], F16)

            # ---- stage 1: stream j-chunks ---------------------------------
            for c in range(NCH):
                fsl = slice(c * FCH, (c + 1) * FCH)

                lg = inp.tile([BA, FCH], F16, tag="lg")
                qt = inp.tile([BA, FCH], F16, tag="qt")
                tq = inp.tile([BA, FCH], F16, tag="tq")
                nc.sync.dma_start(out=lg[:], in_=lg_d[:, fsl])
                nc.sync.dma_start(out=qt[:], in_=qv_d[:, fsl])
                nc.sync.dma_start(out=tq[:], in_=tqv_d[:, fsl])

                e = scr.tile([BA, FCH], F16, tag="e")
                nc.scalar.activation(
                    out=e[:], in_=lg[:], func=mybir.ActivationFunctionType.Exp
                )

                # onehot over the partition-resident n: 4x tensor_scalar
                oh = scr.tile([BA, FCH], F16, tag="oh")
                nc.vector.tensor_scalar(
                    out=oh[:],
                    in0=act_rep[:, fsl],
                    scalar1=iota_c[:],
                    scalar2=None,
                    op0=OP.is_equal,
                )

                peq = scr.tile([BA, FCH], F16, tag="peq")
                nc.vector.tensor_mul(peq[:], e[:], qt[:])
                pel = scr.tile([BA, FCH], F16, tag="pel")
                nc.vector.tensor_mul(pel[:], e[:], lg[:])
                gq = scr.tile([BA, FCH], F16, tag="gq")
                nc.vector.tensor_mul(gq[:], oh[:], qt[:])
                gtq = scr.tile([BA, FCH], F16, tag="gtq")
                nc.vector.tensor_mul(gtq[:], oh[:], tq[:])
                glg = scr.tile([BA, FCH], F16, tag="glg")
                nc.vector.tensor_mul(glg[:], oh[:], lg[:])

                # six reductions accumulate into one [12, SUB] PSUM tile:
                # stationary W_rho[p, m] = 1 iff m == 2*rho + h(p).
                sums_c = sums.tile([12, JCH, T], F16, tag="sums")
                prods = [e, peq, pel, gq, gtq, glg]
                jps = SUB // T  # j's per 512-subchunk
                for s in range(NSUB):
                    ssl = slice(s * SUB, (s + 1) * SUB)
                    ps = ps_red.tile([12, jps, T], F32, tag="red")
                    for rho, p in enumerate(prods):
                        nc.tensor.matmul(
                            out=ps[:],
                            lhsT=wred[:, rho, :],
                            rhs=p[:, ssl],
                            start=(rho == 0),
                            stop=(rho == len(prods) - 1),
                        )
                    nc.scalar.activation(
                        out=sums_c[:, s * jps : (s + 1) * jps, :],
                        in_=ps[:],
                        func=mybir.ActivationFunctionType.Copy,
                    )

                # repack [12, (j, t)] -> s2d[j, 12, t] via a DRAM bounce
                # (a single DMA cannot swap the partition axis with a free
                # axis between two SBUF tiles; DRAM APs are free-form)
                sc = drb.tile([12, JCH, T], F16, tag="sc")
                nc.sync.dma_start(out=sc[:], in_=sums_c[:])
                nc.sync.dma_start(
                    out=s2d[c * JCH : (c + 1) * JCH, :, :],
                    in_=sc[:].transpose([1, 0, 2]),
                )

            # ---- stage 2: merged-h ops on [J, 2, T] slices ----------------
            def S(rho):
                return s2d[:, 2 * rho : 2 * rho + 2, :]

            z = per.tile([J, H, T], F32)
            nc.scalar.activation(
                out=z[:], in_=S(R_SUME), func=mybir.ActivationFunctionType.Ln
            )
            rs = per.tile([J, H, T], F32)
            nc.vector.reciprocal(rs[:], S(R_SUME))

            logp = per.tile([J, H, T], F32)
            nc.vector.tensor_tensor(out=logp[:], in0=S(R_LTK), in1=z[:], op=OP.subtract)
            bl = per.tile([J, H, T], F32)
            nc.vector.tensor_mul(bl[:], S(R_DOTEQ), rs[:])
            adv = per.tile([J, H, T], F32)
            nc.vector.tensor_tensor(out=adv[:], in0=S(R_QTK), in1=bl[:], op=OP.subtract)
            ent = per.tile([J, H, T], F32)
            nc.vector.tensor_mul(ent[:], S(R_DOTEL), rs[:])
            nc.vector.tensor_tensor(out=ent[:], in0=z[:], in1=ent[:], op=OP.subtract)

            pol = per.tile([J, H, T], F32)
            nc.vector.tensor_mul(pol[:], logp[:], adv[:])
            nc.vector.tensor_mul(pol[:], pol[:], w_t[:])
            entw = per.tile([J, H, T], F32)
            nc.vector.tensor_mul(entw[:], ent[:], w_t[:])

            # lambda returns per half: ret[t] = d[t] + g*l*ret[t+1]
            d = per.tile([J, H, T - 1], F32)
            nc.vector.tensor_scalar_mul(
                d[:], S(R_TQTK)[:, :, 1:T], GAMMA * (1.0 - LAMBDA)
            )
            nc.vector.tensor_add(d[:], d[:], r_t[:, :, 0 : T - 1])
            gl = per.tile([J, 1], F32)
            nc.vector.memset(gl[:], GAMMA * LAMBDA)
            ret = per.tile([J, H, T - 1], F32)
            for h in range(H):
                nc.vector.tensor_tensor_scan(
                    out=ret[:, h, ::-1],
                    data0=gl[:].to_broadcast([J, T - 1]),
                    data1=d[:, h, ::-1],
                    initial=s2d[:, 2 * R_TQTK + h, T - 1 : T],
                    op0=OP.mult,
                    op1=OP.add,
                )

            qd = per.tile([J, H, T - 1], F32)
            nc.vector.tensor_tensor(
                out=qd[:], in0=ret[:], in1=S(R_QTK)[:, :, 0 : T - 1], op=OP.subtract
            )
            nc.vector.tensor_mul(qd[:], qd[:], qd[:])
            nc.vector.tensor_mul(qd[:], qd[:], w_t[:, :, 0 : T - 1])

            partials = per.tile([J, H, 3], F32)
            nc.vector.reduce_sum(out=partials[:, :, 0:1], in_=pol[:], axis=AX)
            nc.vector.reduce_sum(out=partials[:, :, 1:2], in_=qd[:], axis=AX)
            nc.vector.reduce_sum(out=partials[:, :, 2:3], in_=entw[:], axis=AX)
            nc.sync.dma_start(out=out_d[:], in_=partials[:])

    return nc


def make_in_maps(logit, action, q_value, target_q_value, reward, weight):
    """Shard + marshal full inputs into per-core input dicts."""
    logit = np.asarray(logit, np.float32)
    q_value = np.asarray(q_value, np.float32)
    target_q_value = np.asarray(target_q_value, np.float32)
    action = np.asarray(action)
    reward = np.asarray(reward, np.float32)
    weight = np.asarray(weight, np.float32)

    iota_c = (np.arange(BA, dtype=np.float32) % J).reshape(BA, 1)
    wred = np.zeros((BA, 6, 12), np.float16)
    for rho in range(6):
        wred[:J, rho, 2 * rho] = 1.0
        wred[J:, rho, 2 * rho + 1] = 1.0
    wred = wred.reshape(BA, 72)

    in_maps = []
    for r in range(M):
        bs, be = r * BL, (r + 1) * BL

        def big(x):
            # [T, BL, A, N] = [t, (h,j), n] -> [h, n, j, t] -> [128, F]
            y = x[:, bs:be].reshape(T, 2, J, N).transpose(1, 3, 2, 0)
            return np.ascontiguousarray(y).reshape(BA, F).astype(np.float16)

        act_c = action[:, bs:be].reshape(T, 2, J)  # [t, h, j]
        # act_rep[64h+n, j*T+t] = action[t, 64h+j]
        act_rep = np.ascontiguousarray(
            np.broadcast_to(
                act_c.transpose(1, 2, 0)[:, None, :, :], (2, N, J, T)
            )
        ).reshape(BA, F).astype(np.float16)

        def small(x):
            # [T, 128] -> [j, h, t]
            y = x.reshape(T, 2, J).transpose(2, 1, 0)
            return np.ascontiguousarray(y).reshape(J, H * T)

        in_maps.append(
            {
                "logit": big(logit),
                "qv": big(q_value),
                "tqv": big(target_q_value),
                "actr": act_rep,
                "iotac": iota_c,
                "wred": wred,
                "wgt": small(weight[:, bs:be].reshape(T, BA)),
                "rwd": small(np.repeat(reward[:, bs:be], A, axis=1)),
            }
        )
    return in_maps


def combine_partials(partials_per_core):
    """[M][64, 6] partial sums -> the three scalar losses."""
    s = np.stack(partials_per_core).astype(np.float64).sum(axis=(0, 1))
    pol = s[0] + s[3]
    qd = s[1] + s[4]
    ent = s[2] + s[5]
    policy_loss = np.float32(-pol / (T * B * A))
    q_value_loss = np.float32(qd / ((T - 1) * B * A))
    entropy_loss = np.float32(ent / (T * B * A))
    return policy_loss, q_value_loss, entropy_loss


_program_cache = {}


def _get_program() -> bass.Bass:
    if "nc" not in _program_cache:
        nc = build_program()
        nc.finalize()
        _program_cache["nc"] = nc
    return _program_cache["nc"]


def kernel(logit, action, q_value, target_q_value, reward, weight):
    nc = _get_program()
    in_maps = make_in_maps(logit, action, q_value, target_q_value, reward, weight)
    res = run_bass_kernel_spmd(nc, in_maps, list(range(M))).results
    return combine_partials(
        [np.asarray(res[i]["out"]).reshape(J, 6) for i in range(M)]
    )


# revision 10
# speedup vs baseline: 1.1694x; 1.0012x over previous
"""COMA loss kernel for Trainium2 — v4: N-on-partition + j-major free dim.

Layout per core (B sharded 8 ways, BL=16, BA=BL*A=128 rows):
  ba = 64*h + j  (h in {0,1}, j in [0,64))
  SBUF partition p = 64*h + n   (n = action index, N=64)
  free index     f = j*T + t    (F = 64*T = 16384), j-major!

All six per-(ba,t) sums over n are PE ones-matmuls (accumulating six
[128,12] one-column stationaries into one [12,512] PSUM tile). Because
f is j-major, the [12, F] sum rows convert to the stage-2 layout
s2d[j, 12, T] with a single strided SBUF->SBUF DMA per chunk (512B
contiguous segments) — no PE transposes at all. Stage-1 streams over
j-blocks of 8 (chunks are f-contiguous). The onehot is a 4x-mode
tensor_scalar is_equal against a per-partition iota. DVE carries only
the five fp16 products plus a compact merged-h stage 2.
"""

import sys

for _p in ("/opt/trn_rl_repo",):
    if _p not in sys.path:
        sys.path.insert(0, _p)

import numpy as np

import concourse.bass as bass
import concourse.bacc as bacc
import concourse.mybir as mybir
from concourse.bass_utils import run_bass_kernel_spmd
from concourse.tile import TileContext

T, B, A, N = 256, 128, 8, 64
M = 8                 # cores
BL = B // M
BA = BL * A           # 128
H, J = 2, 64          # ba = 64h + j
F = J * T             # 16384: f = j*T + t
JCH = 8               # j per chunk
NCH = J // JCH        # 8 chunks
FCH = JCH * T         # 2048
SUB = 512             # matmul f-subchunk (one PSUM bank)
NSUB = FCH // SUB     # 4
GAMMA, LAMBDA = 0.99, 0.95

F32 = mybir.dt.float32
F16 = mybir.dt.float16

# reduction-row order within [12, f]: row = 2*rho + h
R_SUME, R_DOTEQ, R_DOTEL, R_QTK, R_TQTK, R_LTK = range(6)


def build_program() -> bass.Bass:
    nc = bacc.Bacc("TRN2", target_bir_lowering=False, debug=False)

    lg_d = nc.dram_tensor("logit", [BA, F], F16, kind="ExternalInput")
    qv_d = nc.dram_tensor("qv", [BA, F], F16, kind="ExternalInput")
    tqv_d = nc.dram_tensor("tqv", [BA, F], F16, kind="ExternalInput")
    actr_d = nc.dram_tensor("actr", [BA, F], F16, kind="ExternalInput")
    iota_d = nc.dram_tensor("iotac", [BA, 1], F32, kind="ExternalInput")
    wred_d = nc.dram_tensor("wred", [BA, 6 * 12], F16, kind="ExternalInput")
    wgt_d = nc.dram_tensor("wgt", [J, H * T], F16, kind="ExternalInput")
    rwd_d = nc.dram_tensor("rwd", [J, H * T], F16, kind="ExternalInput")
    out_d = nc.dram_tensor("out", [J, H, 3], F32, kind="ExternalOutput")

    OP = mybir.AluOpType
    AX = mybir.AxisListType.X

    with TileContext(nc) as tc:
        with (
            tc.tile_pool(name="inp", bufs=3) as inp,
            tc.tile_pool(name="scr", bufs=2) as scr,
            tc.tile_pool(name="sums", bufs=2) as sums,
            tc.tile_pool(name="per", bufs=1) as per,
            tc.tile_pool(name="ps_red", bufs=6, space=bass.MemorySpace.PSUM) as ps_red,
            tc.tile_pool(name="drb", bufs=2, space="DRAM") as drb,
        ):
            # ---- constants / small inputs ---------------------------------
            iota_c = per.tile([BA, 1], F32)
            nc.sync.dma_start(out=iota_c[:], in_=iota_d[:])
            wred = per.tile([BA, 6, 12], F16)
            nc.sync.dma_start(out=wred[:], in_=wred_d[:])
            w_t = per.tile([J, H, T], F16)
            nc.sync.dma_start(out=w_t[:], in_=wgt_d[:])
            r_t = per.tile([J, H, T], F16)
            nc.sync.dma_start(out=r_t[:], in_=rwd_d[:])
            act_rep = per.tile([BA, F], F16)
            nc.sync.dma_start(out=act_rep[:], in_=actr_d[:])

            # s2d[j, r, t]: per-(ba,t) sums in stage-2 layout, r = 2*rho+h
            s2d = per.tile([J, 12, T], F16)

            # ---- stage 1: stream j-chunks ---------------------------------
            for c in range(NCH):
                fsl = slice(c * FCH, (c + 1) * FCH)

                lg = inp.tile([BA, FCH], F16, tag="lg")
                qt = inp.tile([BA, FCH], F16, tag="qt")
                tq = inp.tile([BA, FCH], F16, tag="tq")
                nc.sync.dma_start(out=lg[:], in_=lg_d[:, fsl])
                nc.sync.dma_start(out=qt[:], in_=qv_d[:, fsl])
                nc.sync.dma_start(out=tq[:], in_=tqv_d[:, fsl])

                e = scr.tile([BA, FCH], F16, tag="e")
                nc.scalar.activation(
                    out=e[:], in_=lg[:], func=mybir.ActivationFunctionType.Exp
                )

                # onehot over the partition-resident n: 4x tensor_scalar
                oh = scr.tile([BA, FCH], F16, tag="oh")
                nc.vector.tensor_scalar(
                    out=oh[:],
                    in0=act_rep[:, fsl],
                    scalar1=iota_c[:],
                    scalar2=None,
                    op0=OP.is_equal,
                )

                gq = scr.tile([BA, FCH], F16, tag="gq")
                nc.vector.tensor_mul(gq[:], oh[:], qt[:])
                gtq = scr.tile([BA, FCH], F16, tag="gtq")
                nc.vector.tensor_mul(gtq[:], oh[:], tq[:])
                glg = scr.tile([BA, FCH], F16, tag="glg")
                nc.vector.tensor_mul(glg[:], oh[:], lg[:])
                peq = scr.tile([BA, FCH], F16, tag="peq")
                nc.vector.tensor_mul(peq[:], e[:], qt[:])
                pel = scr.tile([BA, FCH], F16, tag="pel")
                nc.vector.tensor_mul(pel[:], e[:], lg[:])

                # six reductions accumulate into one [12, SUB] PSUM tile:
                # stationary W_rho[p, m] = 1 iff m == 2*rho + h(p).
                sums_c = sums.tile([12, JCH, T], F16, tag="sums")
                prods = [e, peq, pel, gq, gtq, glg]
                jps = SUB // T  # j's per 512-subchunk
                for s in range(NSUB):
                    ssl = slice(s * SUB, (s + 1) * SUB)
                    ps = ps_red.tile([12, jps, T], F32, tag="red")
                    for rho, p in enumerate(prods):
                        nc.tensor.matmul(
                            out=ps[:],
                            lhsT=wred[:, rho, :],
                            rhs=p[:, ssl],
                            start=(rho == 0),
                            stop=(rho == len(prods) - 1),
                        )
                    nc.scalar.activation(
                        out=sums_c[:, s * jps : (s + 1) * jps, :],
                        in_=ps[:],
                        func=mybir.ActivationFunctionType.Copy,
                    )

                # repack [12, (j, t)] -> s2d[j, 12, t] via a DRAM bounce
                # (a single DMA cannot swap the partition axis with a free
                # axis between two SBUF tiles; DRAM APs are free-form)
                sc = drb.tile([12, JCH, T], F16, tag="sc")
                nc.sync.dma_start(out=sc[:], in_=sums_c[:])
                nc.sync.dma_start(
                    out=s2d[c * JCH : (c + 1) * JCH, :, :],
                    in_=sc[:].transpose([1, 0, 2]),
                )

            # ---- stage 2: merged-h ops on [J, 2, T] slices ----------------
            def S(rho):
                return s2d[:, 2 * rho : 2 * rho + 2, :]

            z = per.tile([J, H, T], F16)
            nc.scalar.activation(
                out=z[:], in_=S(R_SUME), func=mybir.ActivationFunctionType.Ln
            )
            rs = per.tile([J, H, T], F32)
            nc.vector.reciprocal(rs[:], S(R_SUME))

            logp = per.tile([J, H, T], F16)
            nc.vector.tensor_tensor(out=logp[:], in0=S(R_LTK), in1=z[:], op=OP.subtract)
            bl = per.tile([J, H, T], F16)
            nc.vector.tensor_mul(bl[:], S(R_DOTEQ), rs[:])
            adv = per.tile([J, H, T], F16)
            nc.vector.tensor_tensor(out=adv[:], in0=S(R_QTK), in1=bl[:], op=OP.subtract)
            ent = per.tile([J, H, T], F16)
            nc.vector.tensor_mul(ent[:], S(R_DOTEL), rs[:])
            nc.vector.tensor_tensor(out=ent[:], in0=z[:], in1=ent[:], op=OP.subtract)

            pol = per.tile([J, H, T], F16)
            nc.vector.tensor_mul(pol[:], logp[:], adv[:])
            nc.vector.tensor_mul(pol[:], pol[:], w_t[:])
            entw = per.tile([J, H, T], F16)
            nc.vector.tensor_mul(entw[:], ent[:], w_t[:])

            # lambda returns per half: ret[t] = d[t] + g*l*ret[t+1]
            d = per.tile([J, H, T - 1], F16)
            nc.vector.tensor_scalar_mul(
                d[:], S(R_TQTK)[:, :, 1:T], GAMMA * (1.0 - LAMBDA)
            )
            nc.vector.tensor_add(d[:], d[:], r_t[:, :, 0 : T - 1])
            gl = per.tile([J, 1], F16)
            nc.vector.memset(gl[:], GAMMA * LAMBDA)
            ret = per.tile([J, H, T - 1], F16)
            for h in range(H):
                nc.vector.tensor_tensor_scan(
                    out=ret[:, h, ::-1],
                    data0=gl[:].to_broadcast([J, T - 1]),
                    data1=d[:, h, ::-1],
                    initial=s2d[:, 2 * R_TQTK + h, T - 1 : T],
                    op0=OP.mult,
                    op1=OP.add,
                )

            qd = per.tile([J, H, T - 1], F16)
            nc.vector.tensor_tensor(
                out=qd[:], in0=ret[:], in1=S(R_QTK)[:, :, 0 : T - 1], op=OP.subtract
            )
            nc.vector.tensor_mul(qd[:], qd[:], qd[:])
            nc.vector.tensor_mul(qd[:], qd[:], w_t[:, :, 0 : T - 1])

            partials = per.tile([J, H, 3], F32)
            nc.vector.reduce_sum(out=partials[:, :, 0:1], in_=pol[:], axis=AX)
            nc.vector.reduce_sum(out=partials[:, :, 1:2], in_=qd[:], axis=AX)
            nc.vector.reduce_sum(out=partials[:, :, 2:3], in_=entw[:], axis=AX)
            nc.sync.dma_start(out=out_d[:], in_=partials[:])

    return nc


def make_in_maps(logit, action, q_value, target_q_value, reward, weight):
    """Shard + marshal full inputs into per-core input dicts."""
    logit = np.asarray(logit, np.float32)
    q_value = np.asarray(q_value, np.float32)
    target_q_value = np.asarray(target_q_value, np.float32)
    action = np.asarray(action)
    reward = np.asarray(reward, np.float32)
    weight = np.asarray(weight, np.float32)

    iota_c = (np.arange(BA, dtype=np.float32) % J).reshape(BA, 1)
    wred = np.zeros((BA, 6, 12), np.float16)
    for rho in range(6):
        wred[:J, rho, 2 * rho] = 1.0
        wred[J:, rho, 2 * rho + 1] = 1.0
    wred = wred.reshape(BA, 72)

    in_maps = []
    for r in range(M):
        bs, be = r * BL, (r + 1) * BL

        def big(x):
            # [T, BL, A, N] = [t, (h,j), n] -> [h, n, j, t] -> [128, F]
            y = x[:, bs:be].reshape(T, 2, J, N).transpose(1, 3, 2, 0)
            return np.ascontiguousarray(y).reshape(BA, F).astype(np.float16)

        act_c = action[:, bs:be].reshape(T, 2, J)  # [t, h, j]
        # act_rep[64h+n, j*T+t] = action[t, 64h+j]
        act_rep = np.ascontiguousarray(
            np.broadcast_to(
                act_c.transpose(1, 2, 0)[:, None, :, :], (2, N, J, T)
            )
        ).reshape(BA, F).astype(np.float16)

        def small(x):
            # [T, 128] -> [j, h, t]
            y = x.reshape(T, 2, J).transpose(2, 1, 0)
            return np.ascontiguousarray(y).reshape(J, H * T).astype(np.float16)

        in_maps.append(
            {
                "logit": big(logit),
                "qv": big(q_value),
                "tqv": big(target_q_value),
                "actr": act_rep,
                "iotac": iota_c,
                "wred": wred,
                "wgt": small(weight[:, bs:be].reshape(T, BA)),
                "rwd": small(np.repeat(reward[:, bs:be], A, axis=1)),
            }
        )
    return in_maps


def combine_partials(partials_per_core):
    """[M][64, 6] partial sums -> the three scalar losses."""
    s = np.stack(partials_per_core).astype(np.float64).sum(axis=(0, 1))
    pol = s[0] + s[3]
    qd = s[1] + s[4]
    ent = s[2] + s[5]
    policy_loss = np.float32(-pol / (T * B * A))
    q_value_loss = np.float32(qd / ((T - 1) * B * A))
    entropy_loss = np.float32(ent / (T * B * A))
    return policy_loss, q_value_loss, entropy_loss


_program_cache = {}


def _get_program() -> bass.Bass:
    if "nc" not in _program_cache:
        nc = build_program()
        nc.finalize()
        _program_cache["nc"] = nc
    return _program_cache["nc"]


def kernel(logit, action, q_value, target_q_value, reward, weight):
    nc = _get_program()
    in_maps = make_in_maps(logit, action, q_value, target_q_value, reward, weight)
    res = run_bass_kernel_spmd(nc, in_maps, list(range(M))).results
    return combine_partials(
        [np.asarray(res[i]["out"]).reshape(J, 6) for i in range(M)]
    )


# revision 11
# speedup vs baseline: 1.1774x; 1.0068x over previous
"""COMA loss kernel for Trainium2 — v4: N-on-partition + j-major free dim.

Layout per core (B sharded 8 ways, BL=16, BA=BL*A=128 rows):
  ba = 64*h + j  (h in {0,1}, j in [0,64))
  SBUF partition p = 64*h + n   (n = action index, N=64)
  free index     f = j*T + t    (F = 64*T = 16384), j-major!

All six per-(ba,t) sums over n are PE ones-matmuls (accumulating six
[128,12] one-column stationaries into one [12,512] PSUM tile). Because
f is j-major, the [12, F] sum rows convert to the stage-2 layout
s2d[j, 12, T] with a single strided SBUF->SBUF DMA per chunk (512B
contiguous segments) — no PE transposes at all. Stage-1 streams over
j-blocks of 8 (chunks are f-contiguous). The onehot is a 4x-mode
tensor_scalar is_equal against a per-partition iota. DVE carries only
the five fp16 products plus a compact merged-h stage 2.
"""

import sys

for _p in ("/opt/trn_rl_repo",):
    if _p not in sys.path:
        sys.path.insert(0, _p)

import numpy as np

import concourse.bass as bass
import concourse.bacc as bacc
import concourse.mybir as mybir
from concourse.bass_utils import run_bass_kernel_spmd
from concourse.tile import TileContext

T, B, A, N = 256, 128, 8, 64
M = 8                 # cores
BL = B // M
BA = BL * A           # 128
H, J = 2, 64          # ba = 64h + j
F = J * T             # 16384: f = j*T + t
JCH = 8               # j per chunk
NCH = J // JCH        # 8 chunks
FCH = JCH * T         # 2048
SUB = 512             # matmul f-subchunk (one PSUM bank)
NSUB = FCH // SUB     # 4
GAMMA, LAMBDA = 0.99, 0.95

F32 = mybir.dt.float32
F16 = mybir.dt.float16

# reduction-row order within [12, f]: row = 2*rho + h
R_SUME, R_DOTEQ, R_DOTEL, R_QTK, R_TQTK, R_LTK = range(6)


def build_program() -> bass.Bass:
    nc = bacc.Bacc("TRN2", target_bir_lowering=False, debug=False)

    lg_d = nc.dram_tensor("logit", [BA, F], F16, kind="ExternalInput")
    qv_d = nc.dram_tensor("qv", [BA, F], F16, kind="ExternalInput")
    tqv_d = nc.dram_tensor("tqv", [BA, F], F16, kind="ExternalInput")
    actr_d = nc.dram_tensor("actr", [BA, F], F16, kind="ExternalInput")
    iota_d = nc.dram_tensor("iotac", [BA, 1], F32, kind="ExternalInput")
    wred_d = nc.dram_tensor("wred", [BA, 6 * 12], F16, kind="ExternalInput")
    wgt_d = nc.dram_tensor("wgt", [J, H * T], F16, kind="ExternalInput")
    rwd_d = nc.dram_tensor("rwd", [J, H * T], F16, kind="ExternalInput")
    out_d = nc.dram_tensor("out", [J, H, 3], F32, kind="ExternalOutput")

    OP = mybir.AluOpType
    AX = mybir.AxisListType.X

    with TileContext(nc) as tc:
        with (
            tc.tile_pool(name="inp", bufs=3) as inp,
            tc.tile_pool(name="scr", bufs=2) as scr,
            tc.tile_pool(name="sums", bufs=2) as sums,
            tc.tile_pool(name="per", bufs=1) as per,
            tc.tile_pool(name="ps_red", bufs=6, space=bass.MemorySpace.PSUM) as ps_red,
            tc.tile_pool(name="drb", bufs=2, space="DRAM") as drb,
        ):
            # ---- constants / small inputs ---------------------------------
            iota_c = per.tile([BA, 1], F32)
            nc.sync.dma_start(out=iota_c[:], in_=iota_d[:])
            wred = per.tile([BA, 6, 12], F16)
            nc.sync.dma_start(out=wred[:], in_=wred_d[:])
            w_t = per.tile([J, H, T], F16)
            nc.sync.dma_start(out=w_t[:], in_=wgt_d[:])
            r_t = per.tile([J, H, T], F16)
            nc.sync.dma_start(out=r_t[:], in_=rwd_d[:])
            act_rep = per.tile([BA, F], F16)
            nc.sync.dma_start(out=act_rep[:], in_=actr_d[:])

            # s2d[j, r, t]: per-(ba,t) sums in stage-2 layout, r = 2*rho+h
            s2d = per.tile([J, 12, T], F16)

            # ---- stage 1: stream j-chunks ---------------------------------
            for c in range(NCH):
                fsl = slice(c * FCH, (c + 1) * FCH)

                lg = inp.tile([BA, FCH], F16, tag="lg")
                qt = inp.tile([BA, FCH], F16, tag="qt")
                tq = inp.tile([BA, FCH], F16, tag="tq")
                nc.sync.dma_start(out=lg[:], in_=lg_d[:, fsl])
                nc.sync.dma_start(out=qt[:], in_=qv_d[:, fsl])
                nc.sync.dma_start(out=tq[:], in_=tqv_d[:, fsl])

                e = scr.tile([BA, FCH], F16, tag="e")
                nc.scalar.activation(
                    out=e[:], in_=lg[:], func=mybir.ActivationFunctionType.Exp
                )

                # onehot over the partition-resident n: 4x tensor_scalar
                oh = scr.tile([BA, FCH], F16, tag="oh")
                nc.vector.tensor_scalar(
                    out=oh[:],
                    in0=act_rep[:, fsl],
                    scalar1=iota_c[:],
                    scalar2=None,
                    op0=OP.is_equal,
                )

                gq = scr.tile([BA, FCH], F16, tag="gq")
                nc.vector.tensor_mul(gq[:], oh[:], qt[:])
                gtq = scr.tile([BA, FCH], F16, tag="gtq")
                nc.vector.tensor_mul(gtq[:], oh[:], tq[:])
                glg = scr.tile([BA, FCH], F16, tag="glg")
                nc.vector.tensor_mul(glg[:], oh[:], lg[:])
                peq = scr.tile([BA, FCH], F16, tag="peq")
                nc.vector.tensor_mul(peq[:], e[:], qt[:])
                pel = scr.tile([BA, FCH], F16, tag="pel")
                nc.vector.tensor_mul(pel[:], e[:], lg[:])

                # six reductions accumulate into one [12, SUB] PSUM tile:
                # stationary W_rho[p, m] = 1 iff m == 2*rho + h(p).
                sums_c = sums.tile([12, JCH, T], F16, tag="sums")
                prods = [
                    (R_SUME, e),
                    (R_QTK, gq),
                    (R_TQTK, gtq),
                    (R_LTK, glg),
                    (R_DOTEQ, peq),
                    (R_DOTEL, pel),
                ]
                jps = SUB // T  # j's per 512-subchunk
                for s in range(NSUB):
                    ssl = slice(s * SUB, (s + 1) * SUB)
                    ps = ps_red.tile([12, jps, T], F32, tag="red")
                    for i, (rho, p) in enumerate(prods):
                        nc.tensor.matmul(
                            out=ps[:],
                            lhsT=wred[:, rho, :],
                            rhs=p[:, ssl],
                            start=(i == 0),
                            stop=(i == len(prods) - 1),
                        )
                    nc.scalar.activation(
                        out=sums_c[:, s * jps : (s + 1) * jps, :],
                        in_=ps[:],
                        func=mybir.ActivationFunctionType.Copy,
                    )

                # repack [12, (j, t)] -> s2d[j, 12, t] via a DRAM bounce
                # (a single DMA cannot swap the partition axis with a free
                # axis between two SBUF tiles; DRAM APs are free-form)
                sc = drb.tile([12, JCH, T], F16, tag="sc")
                nc.sync.dma_start(out=sc[:], in_=sums_c[:])
                nc.sync.dma_start(
                    out=s2d[c * JCH : (c + 1) * JCH, :, :],
                    in_=sc[:].transpose([1, 0, 2]),
                )

            # ---- stage 2: merged-h ops on [J, 2, T] slices ----------------
            def S(rho):
                return s2d[:, 2 * rho : 2 * rho + 2, :]

            z = per.tile([J, H, T], F16)
            nc.scalar.activation(
                out=z[:], in_=S(R_SUME), func=mybir.ActivationFunctionType.Ln
            )
            rs = per.tile([J, H, T], F32)
            nc.vector.reciprocal(rs[:], S(R_SUME))

            logp = per.tile([J, H, T], F16)
            nc.vector.tensor_tensor(out=logp[:], in0=S(R_LTK), in1=z[:], op=OP.subtract)
            bl = per.tile([J, H, T], F16)
            nc.vector.tensor_mul(bl[:], S(R_DOTEQ), rs[:])
            adv = per.tile([J, H, T], F16)
            nc.vector.tensor_tensor(out=adv[:], in0=S(R_QTK), in1=bl[:], op=OP.subtract)
            ent = per.tile([J, H, T], F16)
            nc.vector.tensor_mul(ent[:], S(R_DOTEL), rs[:])
            nc.vector.tensor_tensor(out=ent[:], in0=z[:], in1=ent[:], op=OP.subtract)

            pol = per.tile([J, H, T], F16)
            nc.vector.tensor_mul(pol[:], logp[:], adv[:])
            nc.vector.tensor_mul(pol[:], pol[:], w_t[:])
            entw = per.tile([J, H, T], F16)
            nc.vector.tensor_mul(entw[:], ent[:], w_t[:])

            # lambda returns per half: ret[t] = d[t] + g*l*ret[t+1]
            d = per.tile([J, H, T - 1], F16)
            nc.vector.tensor_scalar_mul(
                d[:], S(R_TQTK)[:, :, 1:T], GAMMA * (1.0 - LAMBDA)
            )
            nc.vector.tensor_add(d[:], d[:], r_t[:, :, 0 : T - 1])
            gl = per.tile([J, 1], F16)
            nc.vector.memset(gl[:], GAMMA * LAMBDA)
            ret = per.tile([J, H, T - 1], F16)
            for h in range(H):
                nc.vector.tensor_tensor_scan(
                    out=ret[:, h, ::-1],
                    data0=gl[:].to_broadcast([J, T - 1]),
                    data1=d[:, h, ::-1],
                    initial=s2d[:, 2 * R_TQTK + h, T - 1 : T],
                    op0=OP.mult,
                    op1=OP.add,
                )

            qd = per.tile([J, H, T - 1], F16)
            nc.vector.tensor_tensor(
                out=qd[:], in0=ret[:], in1=S(R_QTK)[:, :, 0 : T - 1], op=OP.subtract
            )
            nc.vector.tensor_mul(qd[:], qd[:], qd[:])
            nc.vector.tensor_mul(qd[:], qd[:], w_t[:, :, 0 : T - 1])

            partials = per.tile([J, H, 3], F32)
            nc.vector.reduce_sum(out=partials[:, :, 0:1], in_=pol[:], axis=AX)
            nc.vector.reduce_sum(out=partials[:, :, 1:2], in_=qd[:], axis=AX)
            nc.vector.reduce_sum(out=partials[:, :, 2:3], in_=entw[:], axis=AX)
            nc.sync.dma_start(out=out_d[:], in_=partials[:])

    return nc


def make_in_maps(logit, action, q_value, target_q_value, reward, weight):
    """Shard + marshal full inputs into per-core input dicts."""
    logit = np.asarray(logit, np.float32)
    q_value = np.asarray(q_value, np.float32)
    target_q_value = np.asarray(target_q_value, np.float32)
    action = np.asarray(action)
    reward = np.asarray(reward, np.float32)
    weight = np.asarray(weight, np.float32)

    iota_c = (np.arange(BA, dtype=np.float32) % J).reshape(BA, 1)
    wred = np.zeros((BA, 6, 12), np.float16)
    for rho in range(6):
        wred[:J, rho, 2 * rho] = 1.0
        wred[J:, rho, 2 * rho + 1] = 1.0
    wred = wred.reshape(BA, 72)

    in_maps = []
    for r in range(M):
        bs, be = r * BL, (r + 1) * BL

        def big(x):
            # [T, BL, A, N] = [t, (h,j), n] -> [h, n, j, t] -> [128, F]
            y = x[:, bs:be].reshape(T, 2, J, N).transpose(1, 3, 2, 0)
            return np.ascontiguousarray(y).reshape(BA, F).astype(np.float16)

        act_c = action[:, bs:be].reshape(T, 2, J)  # [t, h, j]
        # act_rep[64h+n, j*T+t] = action[t, 64h+j]
        act_rep = np.ascontiguousarray(
            np.broadcast_to(
                act_c.transpose(1, 2, 0)[:, None, :, :], (2, N, J, T)
            )
        ).reshape(BA, F).astype(np.float16)

        def small(x):
            # [T, 128] -> [j, h, t]
            y = x.reshape(T, 2, J).transpose(2, 1, 0)
            return np.ascontiguousarray(y).reshape(J, H * T).astype(np.float16)

        in_maps.append(
            {
                "logit": big(logit),
                "qv": big(q_value),
                "tqv": big(target_q_value),
                "actr": act_rep,
                "iotac": iota_c,
                "wred": wred,
                "wgt": small(weight[:, bs:be].reshape(T, BA)),
                "rwd": small(np.repeat(reward[:, bs:be], A, axis=1)),
            }
        )
    return in_maps


def combine_partials(partials_per_core):
    """[M][64, 6] partial sums -> the three scalar losses."""
    s = np.stack(partials_per_core).astype(np.float64).sum(axis=(0, 1))
    pol = s[0] + s[3]
    qd = s[1] + s[4]
    ent = s[2] + s[5]
    policy_loss = np.float32(-pol / (T * B * A))
    q_value_loss = np.float32(qd / ((T - 1) * B * A))
    entropy_loss = np.float32(ent / (T * B * A))
    return policy_loss, q_value_loss, entropy_loss


_program_cache = {}


def _get_program() -> bass.Bass:
    if "nc" not in _program_cache:
        nc = build_program()
        nc.finalize()
        _program_cache["nc"] = nc
    return _program_cache["nc"]


def kernel(logit, action, q_value, target_q_value, reward, weight):
    nc = _get_program()
    in_maps = make_in_maps(logit, action, q_value, target_q_value, reward, weight)
    res = run_bass_kernel_spmd(nc, in_maps, list(range(M))).results
    return combine_partials(
        [np.asarray(res[i]["out"]).reshape(J, 6) for i in range(M)]
    )


# revision 12
# speedup vs baseline: 1.3002x; 1.1044x over previous
"""COMA loss kernel for Trainium2 — v4: N-on-partition + j-major free dim.

Layout per core (B sharded 8 ways, BL=16, BA=BL*A=128 rows):
  ba = 64*h + j  (h in {0,1}, j in [0,64))
  SBUF partition p = 64*h + n   (n = action index, N=64)
  free index     f = j*T + t    (F = 64*T = 16384), j-major!

All six per-(ba,t) sums over n are PE ones-matmuls (accumulating six
[128,12] one-column stationaries into one [12,512] PSUM tile). Because
f is j-major, the [12, F] sum rows convert to the stage-2 layout
s2d[j, 12, T] with a single strided SBUF->SBUF DMA per chunk (512B
contiguous segments) — no PE transposes at all. Stage-1 streams over
j-blocks of 8 (chunks are f-contiguous). The onehot is a 4x-mode
tensor_scalar is_equal against a per-partition iota. DVE carries only
the five fp16 products plus a compact merged-h stage 2.
"""

import sys

for _p in ("/opt/trn_rl_repo",):
    if _p not in sys.path:
        sys.path.insert(0, _p)

import numpy as np

import concourse.bass as bass
import concourse.bacc as bacc
import concourse.mybir as mybir
from concourse.bass_utils import run_bass_kernel_spmd
from concourse.tile import TileContext

T, B, A, N = 256, 128, 8, 64
M = 8                 # cores
BL = B // M
BA = BL * A           # 128
H, J = 2, 64          # ba = 64h + j
F = J * T             # 16384: f = j*T + t
JCH = 8               # j per chunk
NCH = J // JCH        # 8 chunks
FCH = JCH * T         # 2048
SUB = 512             # matmul f-subchunk (one PSUM bank)
NSUB = FCH // SUB     # 4
GAMMA, LAMBDA = 0.99, 0.95

F32 = mybir.dt.float32
F16 = mybir.dt.float16

# reduction-row order within [12, f]: row = 2*rho + h
R_SUME, R_DOTEQ, R_DOTEL, R_QTK, R_TQTK, R_LTK = range(6)


def build_program() -> bass.Bass:
    nc = bacc.Bacc("TRN2", target_bir_lowering=False, debug=False)

    lg_d = nc.dram_tensor("logit", [BA, F], F16, kind="ExternalInput")
    qv_d = nc.dram_tensor("qv", [BA, F], F16, kind="ExternalInput")
    tqv_d = nc.dram_tensor("tqv", [BA, F], F16, kind="ExternalInput")
    actr0_d = nc.dram_tensor("actr0", [BA, FCH], F16, kind="ExternalInput")
    actr1_d = nc.dram_tensor("actr1", [BA, F - FCH], F16, kind="ExternalInput")
    iota_d = nc.dram_tensor("iotac", [BA, 1], F32, kind="ExternalInput")
    wred_d = nc.dram_tensor("wred", [BA, 6 * 12], F16, kind="ExternalInput")
    wgt_d = nc.dram_tensor("wgt", [J, H * T], F16, kind="ExternalInput")
    rwd_d = nc.dram_tensor("rwd", [J, H * T], F16, kind="ExternalInput")
    out_d = nc.dram_tensor("out", [J, 3], F32, kind="ExternalOutput")

    OP = mybir.AluOpType
    AX = mybir.AxisListType.X

    with TileContext(nc) as tc:
        with (
            tc.tile_pool(name="inp", bufs=3) as inp,
            tc.tile_pool(name="scr", bufs=3) as scr,
            tc.tile_pool(name="sums", bufs=3) as sums,
            tc.tile_pool(name="per", bufs=1) as per,
            tc.tile_pool(name="ps_red", bufs=6, space=bass.MemorySpace.PSUM) as ps_red,
            tc.tile_pool(name="drb", bufs=2, space="DRAM") as drb,
        ):
            # ---- constants / small inputs ---------------------------------
            iota_c = per.tile([BA, 1], F32)
            nc.sync.dma_start(out=iota_c[:], in_=iota_d[:])
            wred = per.tile([BA, 6, 12], F16)
            nc.sync.dma_start(out=wred[:], in_=wred_d[:])
            w_t = per.tile([J, H, T], F16)
            nc.sync.dma_start(out=w_t[:], in_=wgt_d[:])
            r_t = per.tile([J, H, T], F16)
            nc.sync.dma_start(out=r_t[:], in_=rwd_d[:])
            act_rep0 = per.tile([BA, FCH], F16)
            nc.sync.dma_start(out=act_rep0[:], in_=actr0_d[:])
            act_rep1 = per.tile([BA, F - FCH], F16)

            # s2d[j, r, t]: per-(ba,t) sums in stage-2 layout, r = 2*rho+h
            s2d = per.tile([J, 12, T], F16)

            # ---- stage 1: stream j-chunks ---------------------------------
            for c in range(NCH):
                fsl = slice(c * FCH, (c + 1) * FCH)

                lg = inp.tile([BA, FCH], F16, tag="lg")
                qt = inp.tile([BA, FCH], F16, tag="qt")
                tq = inp.tile([BA, FCH], F16, tag="tq")
                nc.sync.dma_start(out=lg[:], in_=lg_d[:, fsl])
                nc.sync.dma_start(out=qt[:], in_=qv_d[:, fsl])
                nc.sync.dma_start(out=tq[:], in_=tqv_d[:, fsl])
                if c == 0:
                    # bulk of the replicated-action tensor loads after the
                    # first chunk's tensors so chunk-0 compute starts early
                    nc.sync.dma_start(out=act_rep1[:], in_=actr1_d[:])

                e = scr.tile([BA, FCH], F16, tag="e")
                nc.scalar.activation(
                    out=e[:], in_=lg[:], func=mybir.ActivationFunctionType.Exp
                )

                # onehot over the partition-resident n: 4x tensor_scalar
                oh = scr.tile([BA, FCH], F16, tag="oh")
                nc.vector.tensor_scalar(
                    out=oh[:],
                    in0=(
                        act_rep0[:]
                        if c == 0
                        else act_rep1[:, (c - 1) * FCH : c * FCH]
                    ),
                    scalar1=iota_c[:],
                    scalar2=None,
                    op0=OP.is_equal,
                )

                gq = scr.tile([BA, FCH], F16, tag="gq")
                nc.vector.tensor_mul(gq[:], oh[:], qt[:])
                gtq = scr.tile([BA, FCH], F16, tag="gtq")
                nc.vector.tensor_mul(gtq[:], oh[:], tq[:])
                glg = scr.tile([BA, FCH], F16, tag="glg")
                nc.vector.tensor_mul(glg[:], oh[:], lg[:])
                peq = scr.tile([BA, FCH], F16, tag="peq")
                nc.vector.tensor_mul(peq[:], e[:], qt[:])
                pel = scr.tile([BA, FCH], F16, tag="pel")
                nc.vector.tensor_mul(pel[:], e[:], lg[:])

                # six reductions accumulate into one [12, SUB] PSUM tile:
                # stationary W_rho[p, m] = 1 iff m == 2*rho + h(p).
                sums_c = sums.tile([12, JCH, T], F16, tag="sums")
                prods = [
                    (R_SUME, e),
                    (R_QTK, gq),
                    (R_TQTK, gtq),
                    (R_LTK, glg),
                    (R_DOTEQ, peq),
                    (R_DOTEL, pel),
                ]
                jps = SUB // T  # j's per 512-subchunk
                for s in range(NSUB):
                    ssl = slice(s * SUB, (s + 1) * SUB)
                    ps = ps_red.tile([12, jps, T], F32, tag="red")
                    for i, (rho, p) in enumerate(prods):
                        nc.tensor.matmul(
                            out=ps[:],
                            lhsT=wred[:, rho, :],
                            rhs=p[:, ssl],
                            start=(i == 0),
                            stop=(i == len(prods) - 1),
                        )
                    nc.scalar.activation(
                        out=sums_c[:, s * jps : (s + 1) * jps, :],
                        in_=ps[:],
                        func=mybir.ActivationFunctionType.Copy,
                    )

                # repack [12, (j, t)] -> s2d[j, 12, t] via a DRAM bounce
                # (a single DMA cannot swap the partition axis with a free
                # axis between two SBUF tiles; DRAM APs are free-form)
                sc = drb.tile([12, JCH, T], F16, tag="sc")
                nc.sync.dma_start(out=sc[:], in_=sums_c[:])
                nc.sync.dma_start(
                    out=s2d[c * JCH : (c + 1) * JCH, :, :],
                    in_=sc[:].transpose([1, 0, 2]),
                )

            # ---- stage 2: merged-h ops on [J, 2, T] slices ----------------
            def S(rho):
                return s2d[:, 2 * rho : 2 * rho + 2, :]

            # lambda returns per half first: independent of the z/rs chain
            d = per.tile([J, H, T - 1], F16)
            nc.vector.tensor_scalar_mul(
                d[:], S(R_TQTK)[:, :, 1:T], GAMMA * (1.0 - LAMBDA)
            )
            nc.vector.tensor_add(d[:], d[:], r_t[:, :, 0 : T - 1])
            gl = per.tile([J, 1], F16)
            nc.vector.memset(gl[:], GAMMA * LAMBDA)
            ret = per.tile([J, H, T - 1], F16)
            for h in range(H):
                nc.vector.tensor_tensor_scan(
                    out=ret[:, h, ::-1],
                    data0=gl[:].to_broadcast([J, T - 1]),
                    data1=d[:, h, ::-1],
                    initial=s2d[:, 2 * R_TQTK + h, T - 1 : T],
                    op0=OP.mult,
                    op1=OP.add,
                )

            z = per.tile([J, H, T], F16)
            nc.scalar.activation(
                out=z[:], in_=S(R_SUME), func=mybir.ActivationFunctionType.Ln
            )
            se32 = per.tile([J, H, T], F32)
            nc.vector.tensor_copy(se32[:], S(R_SUME))
            rs = per.tile([J, H, T], F32)
            nc.vector.reciprocal_approx_fast(rs[:], se32[:])

            logp = per.tile([J, H, T], F16)
            nc.vector.tensor_tensor(out=logp[:], in0=S(R_LTK), in1=z[:], op=OP.subtract)
            bl = per.tile([J, H, T], F16)
            nc.vector.tensor_mul(bl[:], S(R_DOTEQ), rs[:])
            adv = per.tile([J, H, T], F16)
            nc.vector.tensor_tensor(out=adv[:], in0=S(R_QTK), in1=bl[:], op=OP.subtract)
            ent = per.tile([J, H, T], F16)
            nc.vector.tensor_mul(ent[:], S(R_DOTEL), rs[:])
            nc.vector.tensor_tensor(out=ent[:], in0=z[:], in1=ent[:], op=OP.subtract)

            pol = per.tile([J, H, T], F16)
            nc.vector.tensor_mul(pol[:], logp[:], adv[:])
            nc.vector.tensor_mul(pol[:], pol[:], w_t[:])
            entw = per.tile([J, H, T], F16)
            nc.vector.tensor_mul(entw[:], ent[:], w_t[:])

            qd = per.tile([J, H, T - 1], F16)
            nc.vector.tensor_tensor(
                out=qd[:], in0=ret[:], in1=S(R_QTK)[:, :, 0 : T - 1], op=OP.subtract
            )
            nc.vector.tensor_mul(qd[:], qd[:], qd[:])
            nc.vector.tensor_mul(qd[:], qd[:], w_t[:, :, 0 : T - 1])

            partials = per.tile([J, 3], F32)
            dump = per.tile([J, H, T], F16)
            nc.scalar.activation(
                out=dump[:], in_=pol[:],
                func=mybir.ActivationFunctionType.Copy,
                accum_out=partials[:, 0:1],
            )
            nc.scalar.activation(
                out=dump[:, :, 0 : T - 1], in_=qd[:],
                func=mybir.ActivationFunctionType.Copy,
                accum_out=partials[:, 1:2],
            )
            nc.scalar.activation(
                out=dump[:], in_=entw[:],
                func=mybir.ActivationFunctionType.Copy,
                accum_out=partials[:, 2:3],
            )
            nc.sync.dma_start(out=out_d[:], in_=partials[:])

    return nc


def make_in_maps(logit, action, q_value, target_q_value, reward, weight):
    """Shard + marshal full inputs into per-core input dicts."""
    logit = np.asarray(logit, np.float32)
    q_value = np.asarray(q_value, np.float32)
    target_q_value = np.asarray(target_q_value, np.float32)
    action = np.asarray(action)
    reward = np.asarray(reward, np.float32)
    weight = np.asarray(weight, np.float32)

    iota_c = (np.arange(BA, dtype=np.float32) % J).reshape(BA, 1)
    wred = np.zeros((BA, 6, 12), np.float16)
    for rho in range(6):
        wred[:J, rho, 2 * rho] = 1.0
        wred[J:, rho, 2 * rho + 1] = 1.0
    wred = wred.reshape(BA, 72)

    in_maps = []
    for r in range(M):
        bs, be = r * BL, (r + 1) * BL

        def big(x):
            # [T, BL, A, N] = [t, (h,j), n] -> [h, n, j, t] -> [128, F]
            y = x[:, bs:be].reshape(T, 2, J, N).transpose(1, 3, 2, 0)
            return np.ascontiguousarray(y).reshape(BA, F).astype(np.float16)

        act_c = action[:, bs:be].reshape(T, 2, J)  # [t, h, j]
        # act_rep[64h+n, j*T+t] = action[t, 64h+j]
        act_rep = np.ascontiguousarray(
            np.broadcast_to(
                act_c.transpose(1, 2, 0)[:, None, :, :], (2, N, J, T)
            )
        ).reshape(BA, F).astype(np.float16)

        def small(x):
            # [T, 128] -> [j, h, t]
            y = x.reshape(T, 2, J).transpose(2, 1, 0)
            return np.ascontiguousarray(y).reshape(J, H * T).astype(np.float16)

        in_maps.append(
            {
                "logit": big(logit),
                "qv": big(q_value),
                "tqv": big(target_q_value),
                "actr0": act_rep[:, :FCH].copy(),
                "actr1": act_rep[:, FCH:].copy(),
                "iotac": iota_c,
                "wred": wred,
                "wgt": small(weight[:, bs:be].reshape(T, BA)),
                "rwd": small(np.repeat(reward[:, bs:be], A, axis=1)),
            }
        )
    return in_maps


def combine_partials(partials_per_core):
    """[M][64, 6] partial sums -> the three scalar losses."""
    s = np.stack(partials_per_core).astype(np.float64).sum(axis=(0, 1))
    pol, qd, ent = s[0], s[1], s[2]
    policy_loss = np.float32(-pol / (T * B * A))
    q_value_loss = np.float32(qd / ((T - 1) * B * A))
    entropy_loss = np.float32(ent / (T * B * A))
    return policy_loss, q_value_loss, entropy_loss


_program_cache = {}


def _get_program() -> bass.Bass:
    if "nc" not in _program_cache:
        nc = build_program()
        nc.finalize()
        _program_cache["nc"] = nc
    return _program_cache["nc"]


def kernel(logit, action, q_value, target_q_value, reward, weight):
    nc = _get_program()
    in_maps = make_in_maps(logit, action, q_value, target_q_value, reward, weight)
    res = run_bass_kernel_spmd(nc, in_maps, list(range(M))).results
    return combine_partials(
        [np.asarray(res[i]["out"]).reshape(J, 3) for i in range(M)]
    )


# revision 13
# speedup vs baseline: 1.3417x; 1.0319x over previous
"""COMA loss kernel for Trainium2 — v4: N-on-partition + j-major free dim.

Layout per core (B sharded 8 ways, BL=16, BA=BL*A=128 rows):
  ba = 64*h + j  (h in {0,1}, j in [0,64))
  SBUF partition p = 64*h + n   (n = action index, N=64)
  free index     f = j*T + t    (F = 64*T = 16384), j-major!

All six per-(ba,t) sums over n are PE ones-matmuls (accumulating six
[128,12] one-column stationaries into one [12,512] PSUM tile). Because
f is j-major, the [12, F] sum rows convert to the stage-2 layout
s2d[j, 12, T] with a single strided SBUF->SBUF DMA per chunk (512B
contiguous segments) — no PE transposes at all. Stage-1 streams over
j-blocks of 8 (chunks are f-contiguous). The onehot is a 4x-mode
tensor_scalar is_equal against a per-partition iota. DVE carries only
the five fp16 products plus a compact merged-h stage 2.
"""

import sys

for _p in ("/opt/trn_rl_repo",):
    if _p not in sys.path:
        sys.path.insert(0, _p)

import numpy as np

import concourse.bass as bass
import concourse.bacc as bacc
import concourse.mybir as mybir
from concourse.bass_utils import run_bass_kernel_spmd
from concourse.tile import TileContext

T, B, A, N = 256, 128, 8, 64
M = 8                 # cores
BL = B // M
BA = BL * A           # 128
H, J = 2, 64          # ba = 64h + j
F = J * T             # 16384: f = j*T + t
JCH = 8               # j per chunk
NCH = J // JCH        # 8 chunks
FCH = JCH * T         # 2048
SUB = 512             # matmul f-subchunk (one PSUM bank)
NSUB = FCH // SUB     # 4
GAMMA, LAMBDA = 0.99, 0.95

F32 = mybir.dt.float32
F16 = mybir.dt.float16

# reduction-row order within [12, f]: row = 2*rho + h
R_SUME, R_DOTEQ, R_DOTEL, R_QTK, R_TQTK, R_LTK = range(6)


def build_program() -> bass.Bass:
    nc = bacc.Bacc("TRN2", target_bir_lowering=False, debug=False)

    lg_d = nc.dram_tensor("logit", [BA, F], F16, kind="ExternalInput")
    qv_d = nc.dram_tensor("qv", [BA, F], F16, kind="ExternalInput")
    tqv_d = nc.dram_tensor("tqv", [BA, F], F16, kind="ExternalInput")
    actr0_d = nc.dram_tensor("actr0", [BA, FCH], F16, kind="ExternalInput")
    actr1_d = nc.dram_tensor("actr1", [BA, F - FCH], F16, kind="ExternalInput")
    iota_d = nc.dram_tensor("iotac", [BA, 1], F32, kind="ExternalInput")
    wred_d = nc.dram_tensor("wred", [BA, 6 * 12], F16, kind="ExternalInput")
    wgt_d = nc.dram_tensor("wgt", [J, H * T], F16, kind="ExternalInput")
    rwd_d = nc.dram_tensor("rwd", [J, H * T], F16, kind="ExternalInput")
    out_d = nc.dram_tensor("out", [J, 3], F32, kind="ExternalOutput")

    OP = mybir.AluOpType
    AX = mybir.AxisListType.X

    with TileContext(nc) as tc:
        with (
            tc.tile_pool(name="inp", bufs=3) as inp,
            tc.tile_pool(name="scr", bufs=3) as scr,
            tc.tile_pool(name="sums", bufs=8) as sums,
            tc.tile_pool(name="per", bufs=1) as per,
            tc.tile_pool(name="ps_red", bufs=6, space=bass.MemorySpace.PSUM) as ps_red,
            tc.tile_pool(name="drb", bufs=8, space="DRAM") as drb,
        ):
            # ---- constants / small inputs ---------------------------------
            iota_c = per.tile([BA, 1], F32)
            nc.sync.dma_start(out=iota_c[:], in_=iota_d[:])
            wred = per.tile([BA, 6, 12], F16)
            nc.sync.dma_start(out=wred[:], in_=wred_d[:])
            w_t = per.tile([J, H, T], F16)
            nc.sync.dma_start(out=w_t[:], in_=wgt_d[:])
            r_t = per.tile([J, H, T], F16)
            nc.sync.dma_start(out=r_t[:], in_=rwd_d[:])
            act_rep0 = per.tile([BA, FCH], F16)
            nc.sync.dma_start(out=act_rep0[:], in_=actr0_d[:])
            act_rep1 = per.tile([BA, F - FCH], F16)

            # s2d[j, r, t]: per-(ba,t) sums in stage-2 layout, r = 2*rho+h
            s2d = per.tile([J, 12, T], F16)

            # ---- stage 1: stream j-chunks ---------------------------------
            sums_tiles = []
            for c in range(NCH):
                fsl = slice(c * FCH, (c + 1) * FCH)

                lg = inp.tile([BA, FCH], F16, tag="lg")
                qt = inp.tile([BA, FCH], F16, tag="qt")
                tq = inp.tile([BA, FCH], F16, tag="tq")
                nc.sync.dma_start(out=lg[:], in_=lg_d[:, fsl])
                nc.sync.dma_start(out=qt[:], in_=qv_d[:, fsl])
                nc.sync.dma_start(out=tq[:], in_=tqv_d[:, fsl])
                if c == 0:
                    # bulk of the replicated-action tensor loads after the
                    # first chunk's tensors so chunk-0 compute starts early
                    nc.sync.dma_start(out=act_rep1[:], in_=actr1_d[:])

                e = scr.tile([BA, FCH], F16, tag="e")
                nc.scalar.activation(
                    out=e[:], in_=lg[:], func=mybir.ActivationFunctionType.Exp
                )

                # onehot over the partition-resident n: 4x tensor_scalar
                oh = scr.tile([BA, FCH], F16, tag="oh")
                nc.vector.tensor_scalar(
                    out=oh[:],
                    in0=(
                        act_rep0[:]
                        if c == 0
                        else act_rep1[:, (c - 1) * FCH : c * FCH]
                    ),
                    scalar1=iota_c[:],
                    scalar2=None,
                    op0=OP.is_equal,
                )

                gq = scr.tile([BA, FCH], F16, tag="gq")
                nc.vector.tensor_mul(gq[:], oh[:], qt[:])
                gtq = scr.tile([BA, FCH], F16, tag="gtq")
                nc.vector.tensor_mul(gtq[:], oh[:], tq[:])
                glg = scr.tile([BA, FCH], F16, tag="glg")
                nc.vector.tensor_mul(glg[:], oh[:], lg[:])
                peq = scr.tile([BA, FCH], F16, tag="peq")
                nc.vector.tensor_mul(peq[:], e[:], qt[:])
                pel = scr.tile([BA, FCH], F16, tag="pel")
                nc.vector.tensor_mul(pel[:], e[:], lg[:])

                # six reductions accumulate into one [12, SUB] PSUM tile:
                # stationary W_rho[p, m] = 1 iff m == 2*rho + h(p).
                sums_c = sums.tile([12, JCH, T], F16, tag="sums")
                prods = [
                    (R_SUME, e),
                    (R_QTK, gq),
                    (R_TQTK, gtq),
                    (R_LTK, glg),
                    (R_DOTEQ, peq),
                    (R_DOTEL, pel),
                ]
                jps = SUB // T  # j's per 512-subchunk
                for s in range(NSUB):
                    ssl = slice(s * SUB, (s + 1) * SUB)
                    ps = ps_red.tile([12, jps, T], F32, tag="red")
                    for i, (rho, p) in enumerate(prods):
                        nc.tensor.matmul(
                            out=ps[:],
                            lhsT=wred[:, rho, :],
                            rhs=p[:, ssl],
                            start=(i == 0),
                            stop=(i == len(prods) - 1),
                        )
                    nc.scalar.activation(
                        out=sums_c[:, s * jps : (s + 1) * jps, :],
                        in_=ps[:],
                        func=mybir.ActivationFunctionType.Copy,
                    )

                sums_tiles.append(sums_c)

            # repack [12, (j, t)] -> s2d[j, 12, t] via DRAM bounces, all
            # deferred here so the stream loop's loads own the DMA rings
            # (a single DMA cannot swap the partition axis with a free
            # axis between two SBUF tiles; DRAM APs are free-form)
            for c, sums_c in enumerate(sums_tiles):
                sc = drb.tile([12, JCH, T], F16, tag=f"sc{c}")
                nc.sync.dma_start(out=sc[:], in_=sums_c[:])
                nc.sync.dma_start(
                    out=s2d[c * JCH : (c + 1) * JCH, :, :],
                    in_=sc[:].transpose([1, 0, 2]),
                )

            # ---- stage 2: merged-h ops on [J, 2, T] slices ----------------
            def S(rho):
                return s2d[:, 2 * rho : 2 * rho + 2, :]

            # lambda returns per half first: independent of the z/rs chain
            d = per.tile([J, H, T - 1], F16)
            nc.vector.tensor_scalar_mul(
                d[:], S(R_TQTK)[:, :, 1:T], GAMMA * (1.0 - LAMBDA)
            )
            nc.vector.tensor_add(d[:], d[:], r_t[:, :, 0 : T - 1])
            gl = per.tile([J, 1], F16)
            nc.vector.memset(gl[:], GAMMA * LAMBDA)
            ret = per.tile([J, H, T - 1], F16)
            for h in range(H):
                nc.vector.tensor_tensor_scan(
                    out=ret[:, h, ::-1],
                    data0=gl[:].to_broadcast([J, T - 1]),
                    data1=d[:, h, ::-1],
                    initial=s2d[:, 2 * R_TQTK + h, T - 1 : T],
                    op0=OP.mult,
                    op1=OP.add,
                )

            z = per.tile([J, H, T], F16)
            nc.scalar.activation(
                out=z[:], in_=S(R_SUME), func=mybir.ActivationFunctionType.Ln
            )
            se32 = per.tile([J, H, T], F32)
            nc.vector.tensor_copy(se32[:], S(R_SUME))
            rs = per.tile([J, H, T], F32)
            nc.vector.reciprocal_approx_fast(rs[:], se32[:])

            logp = per.tile([J, H, T], F16)
            nc.vector.tensor_tensor(out=logp[:], in0=S(R_LTK), in1=z[:], op=OP.subtract)
            bl = per.tile([J, H, T], F16)
            nc.vector.tensor_mul(bl[:], S(R_DOTEQ), rs[:])
            adv = per.tile([J, H, T], F16)
            nc.vector.tensor_tensor(out=adv[:], in0=S(R_QTK), in1=bl[:], op=OP.subtract)
            ent = per.tile([J, H, T], F16)
            nc.vector.tensor_mul(ent[:], S(R_DOTEL), rs[:])
            nc.vector.tensor_tensor(out=ent[:], in0=z[:], in1=ent[:], op=OP.subtract)

            pol = per.tile([J, H, T], F16)
            nc.vector.tensor_mul(pol[:], logp[:], adv[:])
            nc.vector.tensor_mul(pol[:], pol[:], w_t[:])
            entw = per.tile([J, H, T], F16)
            nc.vector.tensor_mul(entw[:], ent[:], w_t[:])

            qd = per.tile([J, H, T - 1], F16)
            nc.vector.tensor_tensor(
                out=qd[:], in0=ret[:], in1=S(R_QTK)[:, :, 0 : T - 1], op=OP.subtract
            )
            nc.vector.tensor_mul(qd[:], qd[:], qd[:])
            nc.vector.tensor_mul(qd[:], qd[:], w_t[:, :, 0 : T - 1])

            partials = per.tile([J, 3], F32)
            dump = per.tile([J, H, T], F16)
            nc.scalar.activation(
                out=dump[:], in_=pol[:],
                func=mybir.ActivationFunctionType.Copy,
                accum_out=partials[:, 0:1],
            )
            nc.scalar.activation(
                out=dump[:, :, 0 : T - 1], in_=qd[:],
                func=mybir.ActivationFunctionType.Copy,
                accum_out=partials[:, 1:2],
            )
            nc.scalar.activation(
                out=dump[:], in_=entw[:],
                func=mybir.ActivationFunctionType.Copy,
                accum_out=partials[:, 2:3],
            )
            nc.sync.dma_start(out=out_d[:], in_=partials[:])

    return nc


def make_in_maps(logit, action, q_value, target_q_value, reward, weight):
    """Shard + marshal full inputs into per-core input dicts."""
    logit = np.asarray(logit, np.float32)
    q_value = np.asarray(q_value, np.float32)
    target_q_value = np.asarray(target_q_value, np.float32)
    action = np.asarray(action)
    reward = np.asarray(reward, np.float32)
    weight = np.asarray(weight, np.float32)

    iota_c = (np.arange(BA, dtype=np.float32) % J).reshape(BA, 1)
    wred = np.zeros((BA, 6, 12), np.float16)
    for rho in range(6):
        wred[:J, rho, 2 * rho] = 1.0
        wred[J:, rho, 2 * rho + 1] = 1.0
    wred = wred.reshape(BA, 72)

    in_maps = []
    for r in range(M):
        bs, be = r * BL, (r + 1) * BL

        def big(x):
            # [T, BL, A, N] = [t, (h,j), n] -> [h, n, j, t] -> [128, F]
            y = x[:, bs:be].reshape(T, 2, J, N).transpose(1, 3, 2, 0)
            return np.ascontiguousarray(y).reshape(BA, F).astype(np.float16)

        act_c = action[:, bs:be].reshape(T, 2, J)  # [t, h, j]
        # act_rep[64h+n, j*T+t] = action[t, 64h+j]
        act_rep = np.ascontiguousarray(
            np.broadcast_to(
                act_c.transpose(1, 2, 0)[:, None, :, :], (2, N, J, T)
            )
        ).reshape(BA, F).astype(np.float16)

        def small(x):
            # [T, 128] -> [j, h, t]
            y = x.reshape(T, 2, J).transpose(2, 1, 0)
            return np.ascontiguousarray(y).reshape(J, H * T).astype(np.float16)

        in_maps.append(
            {
                "logit": big(logit),
                "qv": big(q_value),
                "tqv": big(target_q_value),
                "actr0": act_rep[:, :FCH].copy(),
                "actr1": act_rep[:, FCH:].copy(),
                "iotac": iota_c,
                "wred": wred,
                "wgt": small(weight[:, bs:be].reshape(T, BA)),
                "rwd": small(np.repeat(reward[:, bs:be], A, axis=1)),
            }
        )
    return in_maps


def combine_partials(partials_per_core):
    """[M][64, 6] partial sums -> the three scalar losses."""
    s = np.stack(partials_per_core).astype(np.float64).sum(axis=(0, 1))
    pol, qd, ent = s[0], s[1], s[2]
    policy_loss = np.float32(-pol / (T * B * A))
    q_value_loss = np.float32(qd / ((T - 1) * B * A))
    entropy_loss = np.float32(ent / (T * B * A))
    return policy_loss, q_value_loss, entropy_loss


_program_cache = {}


def _get_program() -> bass.Bass:
    if "nc" not in _program_cache:
        nc = build_program()
        nc.finalize()
        _program_cache["nc"] = nc
    return _program_cache["nc"]


def kernel(logit, action, q_value, target_q_value, reward, weight):
    nc = _get_program()
    in_maps = make_in_maps(logit, action, q_value, target_q_value, reward, weight)
    res = run_bass_kernel_spmd(nc, in_maps, list(range(M))).results
    return combine_partials(
        [np.asarray(res[i]["out"]).reshape(J, 3) for i in range(M)]
    )
